# revision 5
# baseline (speedup 1.0000x reference)
"""Locally-connected Conv2d (nn.Conv2dLocal) Trainium2 Bass kernel.

Problem (hardcoded):
  x:      [B=64, C=64, H=32, W=32]  f32
  weight: [OH=32, OW=32, O=64, C=64, KH=3, KW=3] f32
  bias:   [O=64, OH=32, OW=32] f32
  out:    [B=64, O=64, OH=32, OW=32] f32
  out[b,o,oh,ow] = bias[o,oh,ow]
      + sum_{c,kh,kw} x[b,c,oh+kh-1,ow+kw-1] * weight[oh,ow,o,c,kh,kw]

Sharding: 8 cores, core i owns output rows oh in [4i, 4i+4).

Per-core compute layout:
  - padded input slab rows r = 0..5 (padded coords), cols iw = 0..33.
  - x strips: strip p = rows (p, p+1), SBUF [128=(row?c), 34*64=(iw,b)].
  - matmul: out_loc[b, o] += lhsT.T @ rhs with
      lhsT = x strip slice [K, 64=b] (K=128 for kh=0,1 pairs, K=64 for kh=2)
      rhs  = weight tile   [K, n_ow*64=(ow,o)]
    accumulating into a PSUM bank per (quarter, oh): [64=b, 512=(ow8,o)].
  - bias added during PSUM->SBUF evacuation (DVE tensor_add).
"""

import numpy as np

B, C, H, W = 64, 64, 32, 32
O, KH, KW = 64, 3, 3
NCORES = 8
RPC = 4              # output rows per core
SLAB = RPC + 2       # padded input rows per core
PW = W + 2           # padded width (34)
NQ = 4               # ow quarters
QW = 8               # ow per quarter

_cache = {}


def _sched():
    """Static per-core schedule, identical for every core.

    Returns (chunks, total_cols). Each chunk is one weight DMA per
    (quarter q, pixel column iw) holding 6 weight tiles:
      tiles 0..3: kh={0,1} pair for oh=0..3   (K=128 rows = (kh, c))
      tile  4   : kh=2 for oh=1 (rows 0:64) and oh=0 (rows 64:128)
      tile  5   : kh=2 for oh=2 (rows 0:64) and oh=3 (rows 64:128)
    Each tile spans columns (ow asc, o) for ow in `ows`.
    MM descriptors: (oh, strip, p0, psz, tile_idx, tile_p0).
    """
    chunks = []
    off = 0
    for q in range(NQ):
        for iw in range(QW * q, QW * q + QW + 2):
            ows = [ow for ow in (iw - 2, iw - 1, iw) if QW * q <= ow < QW * q + QW]
            n = len(ows) * O
            mms = [
                # kh01 MMs, full K=128
                (0, 0, 0, 128, 0, 0),
                (1, 1, 0, 128, 1, 0),
                (2, 2, 0, 128, 2, 0),
                (3, 3, 0, 128, 3, 0),
                # kh2 MMs, K=64 (strip row = oh+2)
                (1, 3, 0, 64, 4, 0),     # row 3 = strip3 top
                (0, 1, 64, 64, 4, 64),   # row 2 = strip1 bottom
                (2, 4, 0, 64, 5, 0),     # row 4 = strip4 top
                (3, 4, 64, 64, 5, 64),   # row 5 = strip4 bottom
            ]
            chunks.append(dict(q=q, iw=iw, ows=ows, n=n, off=off, mms=mms))
            off += 6 * n
    return chunks, off


def _host_arrays(x, weight, bias):
    """Build per-core input dicts (all DMA-contiguous layouts)."""
    chunks, total = _sched()
    xp = np.pad(x, ((0, 0), (0, 0), (1, 1), (1, 1)))
    in_maps = []
    for i in range(NCORES):
        slab = xp[:, :, RPC * i:RPC * i + SLAB, :]          # [B, C, 6, 34]
        xs = np.stack([
            np.ascontiguousarray(
                slab[:, :, p:p + 2, :].transpose(2, 1, 3, 0).reshape(128, PW * B))
            for p in range(SLAB - 1)
        ])                                                   # [5, 128, 2176]
        w4 = weight[RPC * i:RPC * i + RPC]                   # [4, 32, O, C, 3, 3]
        ws = np.empty((128, total), dtype=np.float32)
        for ch in chunks:
            iw, ows, n, off = ch["iw"], ch["ows"], ch["n"], ch["off"]
            cols = []
            for oh in range(4):                              # tiles 0..3 (kh01)
                blocks = [
                    w4[oh, ow, :, :, 0:2, iw - ow].transpose(2, 1, 0).reshape(128, O)
                    for ow in ows
                ]
                cols.append(np.concatenate(blocks, axis=1))
            for top_oh, bot_oh in ((1, 0), (2, 3)):          # tiles 4, 5 (kh2)
                top = np.concatenate(
                    [w4[top_oh, ow, :, :, 2, iw - ow].T for ow in ows], axis=1)
                bot = np.concatenate(
                    [w4[bot_oh, ow, :, :, 2, iw - ow].T for ow in ows], axis=1)
                cols.append(np.concatenate([top, bot], axis=0))
            ws[:, off:off + 6 * n] = np.concatenate(cols, axis=1)
        # bias: [O, 4, 32] slab -> per-q bf16 hi/lo rows [NQ, 1, 2*2048]
        import ml_dtypes
        b4 = bias[:, RPC * i:RPC * i + RPC, :].transpose(1, 2, 0)  # [oh, ow, o]
        bse = np.empty((NQ, 1, 2 * RPC * QW * O), dtype=ml_dtypes.bfloat16)
        for q in range(NQ):
            flat = np.ascontiguousarray(
                b4[:, QW * q:QW * q + QW, :]).reshape(-1)    # [4*8*64] f32
            hi = flat.astype(ml_dtypes.bfloat16)
            lo = (flat - hi.astype(np.float32)).astype(ml_dtypes.bfloat16)
            bse[q, 0, :flat.size] = hi
            bse[q, 0, flat.size:] = lo
        in_maps.append({
            "xs": np.ascontiguousarray(xs, dtype=np.float32),
            "ws": ws,
            "bse": bse,
        })
    return in_maps


def _build_program():
    from contextlib import ExitStack
    import concourse.bass as bass
    import concourse.bacc as bacc
    import concourse.tile as tile
    from concourse import mybir

    F32 = mybir.dt.float32
    BF16 = mybir.dt.bfloat16
    chunks, total = _sched()

    nc = bacc.Bacc("TRN2", target_bir_lowering=False, debug=False,
                   num_devices=NCORES)
    xs_d = nc.dram_tensor("xs", [SLAB - 1, 128, PW * B], F32, kind="ExternalInput")
    ws_d = nc.dram_tensor("ws", [128, total], F32, kind="ExternalInput")
    bse_d = nc.dram_tensor("bse", [NQ, 1, 2 * RPC * QW * O], BF16,
                           kind="ExternalInput")
    out_d = nc.dram_tensor("out", [B, RPC * W * O], F32, kind="ExternalOutput")

    # stop flag on the last MM per (q, oh) bank group (start is the bias MM)
    laststop = set()
    for q in range(NQ):
        seen = {}
        for ci, ch in enumerate(chunks):
            if ch["q"] != q:
                continue
            for mi, mm in enumerate(ch["mms"]):
                seen.setdefault(mm[0], []).append((ci, mi))
        for oh, lst in seen.items():
            laststop.add(lst[-1])

    with ExitStack() as ctx:
        tc = ctx.enter_context(tile.TileContext(nc))
        xpool = ctx.enter_context(tc.tile_pool(name="xs", bufs=SLAB - 1))
        wpool = ctx.enter_context(tc.tile_pool(name="wt", bufs=4))
        bpool = ctx.enter_context(tc.tile_pool(name="bias", bufs=2))
        opool = ctx.enter_context(tc.tile_pool(name="outs", bufs=4))
        pspool = ctx.enter_context(
            tc.tile_pool(name="ps", bufs=8, space=bass.MemorySpace.PSUM))

        ones = xpool.tile([1, B], BF16, tag="ones", name="ones")
        nc.gpsimd.memset(ones[:], 1.0)
        strips = []
        for p in range(SLAB - 1):
            xst = xpool.tile([128, PW * B], F32, tag="xstrip", name=f"xstrip{p}")
            nc.sync.dma_start(xst[:], xs_d[p])
            strips.append(xst)

        ws_ap = ws_d.ap()
        out_ap = out_d.ap()
        QO = QW * O  # 512, one psum bank
        for q in range(NQ):
            bt = bpool.tile([1, 2 * RPC * QO], BF16, tag="bias", name=f"bias{q}")
            ps = [pspool.tile([B, QO], F32, tag="psb", name=f"ps{q}_{oh}")
                  for oh in range(RPC)]
            nc.sync.dma_start(bt[:], bse_d[q])
            # bias init: rank-1 (ones x bias) matmuls, hi then lo; hi opens
            # the accumulation group over the full bank.
            for oh in range(RPC):
                nc.tensor.matmul(ps[oh][:, 0:QO], ones[:],
                                 bt[0:1, oh * QO:(oh + 1) * QO],
                                 start=True, stop=False)
                nc.tensor.matmul(ps[oh][:, 0:QO], ones[:],
                                 bt[0:1, RPC * QO + oh * QO:RPC * QO + (oh + 1) * QO],
                                 start=False, stop=False)
            for ci, ch in enumerate(chunks):
                if ch["q"] != q:
                    continue
                iw, ows, n, off = ch["iw"], ch["ows"], ch["n"], ch["off"]
                wt = wpool.tile([128, 6 * n], F32, tag="wtile",
                                name=f"wt{q}_{iw}")
                nc.sync.dma_start(wt[:], ws_ap[:, off:off + 6 * n])
                c0 = (ows[0] - QW * q) * O
                for mi, mm in enumerate(ch["mms"]):
                    oh, sp, p0, psz, ti, tp0 = mm
                    nc.tensor.matmul(
                        ps[oh][:, c0:c0 + n],
                        strips[sp][p0:p0 + psz, iw * B:(iw + 1) * B],
                        wt[tp0:tp0 + psz, ti * n:ti * n + n],
                        start=False, stop=(ci, mi) in laststop,
                    )
            for oh in range(RPC):
                ot = opool.tile([B, QO], F32, tag="ot", name=f"ot{q}_{oh}")
                nc.scalar.copy(ot[:], ps[oh][:])
                nc.sync.dma_start(
                    out_ap[:, (oh * W + q * QW) * O:(oh * W + q * QW) * O + QO],
                    ot[:])

    nc.compile()
    return nc


def kernel(x, weight, bias):
    x = np.asarray(x, dtype=np.float32)
    weight = np.asarray(weight, dtype=np.float32)
    bias = np.asarray(bias, dtype=np.float32)

    from concourse.bass_utils import run_bass_kernel_spmd

    if "nc" not in _cache:
        _cache["nc"] = _build_program()
    nc = _cache["nc"]

    in_maps = _host_arrays(x, weight, bias)
    res = run_bass_kernel_spmd(nc, in_maps, list(range(NCORES)))
    out = np.empty((B, O, H, W), dtype=np.float32)
    for i in range(NCORES):
        o_i = res.results[i]["out"].reshape(B, RPC, W, O)   # [b, oh_l, ow, o]
        out[:, :, RPC * i:RPC * i + RPC, :] = o_i.transpose(0, 3, 1, 2)
    return out
